# revision 38
# baseline (speedup 1.0000x reference)
"""ALiBi multi-head causal attention on 8 TRN2 NeuronCores.

Sharding: core c handles batch b = c // 4 and a group of 4 heads chosen by
the host. Fully data/head-parallel: no collectives; host scatters inputs and
concatenates per-core outputs.

ALiBi block sparsity: for slope m, softmax weights vanish once
|m * (j - i)| exceeds MARGIN. Each head only needs a prefix of key blocks
(m < 0: j < P*128) and/or a diagonal band (m > 0: i - 128*A <= j <= i).
The host groups the 16 heads into 4 NEFF slot positions (union pattern per
slot, one head per slot on each core) and compiles a block-sparse schedule
(cached per pattern).

Host prep: x is pre-transposed per batch (xT [D, S]) so the kernel never
transposes on-chip; W is packed per-core into 128-col groups
[Q0K0 | Q1K1 | Q2Q3 | K2K3 | V0V1 | V2V3] so K of prefix-only slots (2,3)
is projected only for the k-blocks those slots attend (kb < P).

Per-core device algorithm (bf16 operands, fp32 PSUM accumulation):
  - Projection transposed per group: pps = (W-group)^T @ x^T, rows 0:64 and
    64:128 copied (Vector/GpSimd) into qt/kt feature tiles.
  - Scores in transposed layout sT[j, i]: prefix k-blocks as 512-wide
    matmuls spanning 4 q-blocks, band blocks as 128-wide units; units are
    packed into 2-bank PSUM tiles so a single ScalarE exp covers <=1024
    cols. The softmax exponent q.k/32 + m*(j-i) - C is produced by the QK
    matmul via six extra contraction rows (bf16 hi/lo/lo2 triples):
       qt rows 64-66 = split(-relu(m)*i - C), rows 67-69 = 1
       kt rows 64-66 = 1,                     rows 67-69 = split(m*j)
    so exp() needs no max pass.
  - Causal mask: diagonal units are packed adjacently so one gpsimd
    affine_select covers a run of them (post-exp, fill 0).
  - PV in natural layout: out[q, hd] accumulated in PSUM with e-blocks
    stationary; V pair-tiles [128, kb, 2, 65] carry a ones column at
    feature 0 so PV col 0 is the softmax denominator; batch-reciprocal +
    per-block scale into a bf16 staging tile, DMA out (host upcasts).
"""

import ml_dtypes
import numpy as np

import concourse.bass as bass
import concourse.mybir as mybir
import concourse.tile as tile
from concourse import bacc
from concourse.bass_utils import run_bass_kernel_spmd

F32 = mybir.dt.float32
BF16 = mybir.dt.bfloat16

B, S, D, H, HD = 2, 2048, 1024, 16, 64
HPC = 4  # heads per core
N_CORES = 8
C_STAB = 8.0
SCALE32 = 32.0
NDC = D // 128  # 8 contraction chunks
NKB = S // 128  # 16 k-blocks
NQB = S // 128  # 16 q-blocks
MARGIN = 12.0  # exponent cutoff for block skipping (weights < e^-12 dropped)
QK_TILE = 512  # score cols per PSUM score tile / per exp

# NOTE: score PSUM tiles must be single-bank (QK_TILE=512): 2-bank tiles
# with multiple start=True matmuls per bank corrupt values on hardware
# (sim-clean, HW-NaN under profiling timing).
# NOTE: tile dependency tracking is whole-tile granular, so x^T and W live
# in per-chunk tiles (a single big tile makes the first matmul wait for
# every DMA issued before it).
PAD_EXTRAS = False  # host-pad eq/ek to 64 rows (no on-chip memset of qt/kt)
DUMMY_EXP = True  # prefetch exp table via dummy activation at t=0
VT2_SINGLE_COPY = True  # single strided copy into V pair tile
GROUPED_AFFINE = True  # one affine_select per run of adjacent diag blocks

_NC_CACHE = {}


def _head_pattern(mh):
    """(P, A): prefix blocks (m<0) and band-back blocks (m>0); A=-1 none."""
    am = abs(float(mh))
    if am < MARGIN / S:
        return (NKB, -1) if mh < 0 else (0, NKB)
    nb = int(np.ceil(MARGIN / am / 128.0))
    if mh < 0:
        return (min(NKB, nb), -1)
    return (0, min(NKB, nb))


def _kb_set(pat, qb):
    P, A = pat
    s = set()
    if P > 0:
        s |= set(range(0, min(qb, P - 1) + 1))
    if A >= 0:
        s |= set(range(max(0, qb - A), qb + 1))
    return sorted(s)


def _pat_cost(pat):
    return sum(len(_kb_set(pat, qb)) for qb in range(NQB))


def _union(pats):
    P = max(p for p, _ in pats) if pats else 0
    A = max(a for _, a in pats) if pats else -1
    return (P, A)


def _kv_blocks(pat):
    need = set()
    for qb in range(NQB):
        need |= set(_kb_set(pat, qb))
    return need


def _grouping_score(cols, pats):
    ups = [_union([pats[x] for x in c]) for c in cols]
    atten = sum(_pat_cost(u) for u in ups)
    order = sorted(range(4), key=lambda ci: -_pat_cost(ups[ci]))
    ups_o = [ups[i] for i in order]
    vp = len(_kv_blocks(ups_o[0]) | _kv_blocks(ups_o[1])) + len(
        _kv_blocks(ups_o[2]) | _kv_blocks(ups_o[3])
    )
    return atten + 2 * vp


def _group_heads(m16):
    """Group 16 heads into 4 slot columns of 4, minimizing union cost.
    Returns (columns, slot_pats) with columns sorted heavy-first."""
    pats = [_head_pattern(m16[h]) for h in range(H)]

    cands = []
    # (i) greedy by attention delta
    order = sorted(range(H), key=lambda h: -_pat_cost(pats[h]))
    cols = [[] for _ in range(4)]
    for h in order:
        best, bestd = None, None
        for ci in range(4):
            if len(cols[ci]) >= 4:
                continue
            cur = _pat_cost(_union([pats[x] for x in cols[ci]])) if cols[ci] else 0
            new = _pat_cost(_union([pats[x] for x in cols[ci]] + [pats[h]]))
            d = new - cur
            if bestd is None or d < bestd or (
                d == bestd and len(cols[ci]) < len(cols[best])
            ):
                best, bestd = ci, d
        cols[best].append(h)
    cands.append([list(c) for c in cols])
    # (ii) structured by sign
    negs = sorted([h for h in range(H) if pats[h][1] < 0], key=lambda h: pats[h][0])
    poss = sorted([h for h in range(H) if pats[h][1] >= 0], key=lambda h: -pats[h][1])
    seq = negs[: (len(negs) // 4) * 4]
    cols2 = [seq[i * 4 : (i + 1) * 4] for i in range(len(seq) // 4)]
    rest = negs[(len(negs) // 4) * 4 :] + poss
    for i in range(0, len(rest), 4):
        cols2.append(rest[i : i + 4])
    if len(cols2) == 4 and all(len(c) == 4 for c in cols2):
        cands.append(cols2)
    best_cols = min(cands, key=lambda c: _grouping_score(c, pats))
    improved = True
    while improved:
        improved = False
        sc0 = _grouping_score(best_cols, pats)
        for a in range(4):
            for b in range(a + 1, 4):
                for i in range(4):
                    for j in range(4):
                        best_cols[a][i], best_cols[b][j] = (
                            best_cols[b][j], best_cols[a][i],
                        )
                        sc = _grouping_score(best_cols, pats)
                        if sc < sc0:
                            sc0 = sc
                            improved = True
                        else:
                            best_cols[a][i], best_cols[b][j] = (
                                best_cols[b][j], best_cols[a][i],
                            )
    cols = best_cols
    slot_pats = [_union([pats[x] for x in c]) for c in cols]
    idx = sorted(range(4), key=lambda ci: -_pat_cost(slot_pats[ci]))
    cols = [cols[i] for i in idx]
    slot_pats = [slot_pats[i] for i in idx]
    return cols, tuple(slot_pats)


def _narrow_blocks(slot_pats, slot, sg):
    """Band blocks of an s-group not covered by the wide prefix units."""
    P, _ = slot_pats[slot]
    out = []
    for qb in range(4 * sg, 4 * sg + 4):
        for kb in _kb_set(slot_pats[slot], qb):
            if kb < P:
                continue  # covered by wide prefix unit
            out.append((qb, kb))
    return out


def _build_nc(slot_pats):
    nc = bacc.Bacc(None, target_bir_lowering=False, debug=False)
    nex = 64 if PAD_EXTRAS else 6
    xt_ext = nc.declare_dram_parameter("xt", [D, S], BF16, isOutput=False)
    w_ext = nc.declare_dram_parameter("w", [128, 6, NDC, 128], BF16, isOutput=False)
    eq_ext = nc.declare_dram_parameter("eq", [HPC, nex, S], BF16, isOutput=False)
    ek_ext = nc.declare_dram_parameter("ek", [HPC, nex, S], BF16, isOutput=False)
    out_ext = nc.declare_dram_parameter("out", [S, HPC * HD], BF16, isOutput=True)

    with tile.TileContext(nc) as tc:
        _emit(nc, tc, xt_ext, w_ext, eq_ext, ek_ext, out_ext, slot_pats)
    nc.finalize()
    return nc


def _emit(nc, tc, xt_ext, w_ext, eq_ext, ek_ext, out_ext, slot_pats):
    AF = mybir.ActivationFunctionType
    OP = mybir.AluOpType

    persist = tc.alloc_tile_pool(name="persist", bufs=1, space="SBUF")

    # --- persistent SBUF tiles ---
    # per-group W and per-(dc, sg) x^T tiles: fine-grained DMA deps
    w_g = [
        persist.tile([128, NDC, 128], BF16, tag=f"w{g}", name=f"w{g}")
        for g in range(6)
    ]
    xts = [
        [
            persist.tile([128, 512], BF16, tag=f"xt{dc}_{sg}", name=f"xt{dc}_{sg}")
            for sg in range(4)
        ]
        for dc in range(NDC)
    ]
    out_stage = persist.tile([128, 8, HPC * HD], BF16, tag="out_stage")

    # wide-prefix slots (P>=1) and per-slot K extent
    P_of = [slot_pats[s][0] for s in range(HPC)]
    full_k = [s for s in range(HPC) if slot_pats[s][1] >= 0 or P_of[s] >= NKB]
    # K tiles: full-band slots need all S cols; prefix-only slots need P*128
    qt, kt = [], []
    for s in range(HPC):
        qt_s = persist.tile([128, S], BF16, tag=f"qt{s}", name=f"qt{s}")
        kcols = S if s in full_k else max(1, P_of[s]) * 128
        kt_s = persist.tile([128, kcols], BF16, tag=f"kt{s}", name=f"kt{s}")
        qt.append(qt_s)
        kt.append(kt_s)

    # V pair tiles: [128, kb, parity, 65]; feature col 0 is the ones column
    # (softmax denominator accumulator), cols 1:65 the 64 head features.
    slot_kv = [_kv_blocks(slot_pats[s]) for s in range(HPC)]
    pair_kv = [sorted(slot_kv[0] | slot_kv[1]), sorted(slot_kv[2] | slot_kv[3])]
    vt2 = []
    for vp in range(2):
        nkb = max(pair_kv[vp]) + 1
        v_t = persist.tile([128, nkb, 2, 65], BF16, tag=f"vt{vp}", name=f"vt{vp}")
        nc.gpsimd.memset(v_t[:, :, :, 0:1], 1.0)
        vt2.append(v_t)

    dummy = persist.tile([128, 8], F32, tag="dummy")

    # PSUM -> SBUF copies: GpSimd cannot access PSUM.  Steady-state copies
    # ride VectorE; the sg0 prologue copies ride the then-idle ScalarE.
    def copy_any(out, in_):
        nc.vector.tensor_copy(out=out, in_=in_)

    def copy_scalar(out, in_):
        nc.scalar.copy(out=out, in_=in_)

    mix_state = [0]

    def copy_mix(out, in_):
        # prologue copies alternate Scalar/Vector so each PSUM tile is
        # released after ~one copy latency instead of two in FIFO
        if mix_state[0] % 2 == 0:
            nc.scalar.copy(out=out, in_=in_)
        else:
            nc.vector.tensor_copy(out=out, in_=in_)
        mix_state[0] += 1

    with (
        tc.tile_pool(name="pp_ps", bufs=2, space="PSUM") as pp_pool,
        tc.tile_pool(name="qk_ps", bufs=2048 // QK_TILE, space="PSUM")
        as qk_pool,
        tc.tile_pool(name="pv_ps", bufs=2, space="PSUM") as pv_pool,
        tc.tile_pool(
            name="e_pool", bufs=10 * 1024 // QK_TILE, space="SBUF"
        ) as e_pool,
        tc.tile_pool(name="n_pool", bufs=4, space="SBUF") as n_pool,
    ):
        # ---------- DMA emitters ----------
        def emit_w_dma(g):
            nc.sync.dma_start(out=w_g[g], in_=w_ext[:, g])

        def emit_xt_dma(sg):
            for dc in range(NDC):
                nc.sync.dma_start(
                    out=xts[dc][sg],
                    in_=xt_ext[128 * dc : 128 * dc + 128, 512 * sg : 512 * sg + 512],
                )

        nex = 64 if PAD_EXTRAS else 6

        def emit_extras(s):
            nc.sync.dma_start(out=qt[s][64 : 64 + nex, :], in_=eq_ext[s])
            kc = kt[s].shape[-1]
            nc.sync.dma_start(
                out=kt[s][64 : 64 + nex, 0:kc], in_=ek_ext[s, :, 0:kc]
            )

        # ---------- projection ----------
        # full-sequence projection groups: Q always; K-halves of groups 0,1
        # only when that slot attends everywhere; group 3 (K2 K3) only when
        # one of slots 2,3 does.
        proj_groups = [0, 1, 2] + (
            [3] if (2 in full_k or 3 in full_k) else []
        )

        def proj_thunks(sg, cp=None):
            """Per s-group projection of the full-sequence column groups."""
            cp = cp or copy_any
            thunks = []
            for g in proj_groups:
                holder = {}
                for dc in range(NDC):
                    def t_mm(g=g, sg=sg, holder=holder, dc=dc):
                        if "pps" not in holder:
                            holder["pps"] = pp_pool.tile(
                                [128, 512], F32, tag="pp", name=f"pps_{g}_{sg}"
                            )
                        nc.tensor.matmul(
                            holder["pps"],
                            lhsT=w_g[g][:, dc, :],
                            rhs=xts[dc][sg],
                            start=(dc == 0), stop=(dc == NDC - 1),
                        )
                    thunks.append(t_mm)

                def t_cp(g=g, sg=sg, holder=holder, cp=cp):
                    pps = holder["pps"]
                    c0 = 512 * sg
                    if g < 2:
                        cp(qt[g][0:64, c0 : c0 + 512], pps[0:64, :])
                        if g in full_k:
                            cp(kt[g][0:64, c0 : c0 + 512], pps[64:128, :])
                    elif g == 2:
                        cp(qt[2][0:64, c0 : c0 + 512], pps[0:64, :])
                        cp(qt[3][0:64, c0 : c0 + 512], pps[64:128, :])
                    else:
                        for s in (2, 3):
                            if s in full_k:
                                cp(
                                    kt[s][0:64, c0 : c0 + 512],
                                    pps[64 * (s - 2) : 64 * (s - 2) + 64, :],
                                )
                thunks.append(t_cp)
            return thunks

        def emit_kprefix():
            """K for prefix-only slots: only their first P k-blocks."""
            # group 3 carries K2|K3; groups 0,1 carry their own K-half
            jobs = {}  # (grp, kb) -> [(slot, rows0)]
            for s in range(HPC):
                if s in full_k:
                    continue
                g = s if s < 2 else 3
                r0 = 64 if s < 2 else 64 * (s - 2)
                for kb in range(P_of[s]):
                    jobs.setdefault((g, kb), []).append((s, r0))
            for (g, kb), dests in sorted(jobs.items()):
                pps = pp_pool.tile([128, 128], F32, tag="pp", name=f"kp_{g}_{kb}")
                for dc in range(NDC):
                    nc.tensor.matmul(
                        pps,
                        lhsT=w_g[g][:, dc, :],
                        rhs=xts[dc][kb // 4][:, 128 * (kb % 4) : 128 * (kb % 4) + 128],
                        start=(dc == 0), stop=(dc == NDC - 1),
                    )
                for (s, r0) in dests:
                    copy_scalar(
                        kt[s][0:64, 128 * kb : 128 * kb + 128],
                        pps[r0 : r0 + 64, :],
                    )

        # ---------- V projection ----------
        def v_thunks(sg):
            out = []
            for vp in range(2):
                for kb in range(4 * sg, 4 * sg + 4):
                    if kb not in pair_kv[vp]:
                        continue

                    def tv(vp=vp, kb=kb):
                        vps = pp_pool.tile(
                            [128, 128], F32, tag="pp", name=f"vps_{vp}_{kb}"
                        )
                        for dc in range(NDC):
                            nc.tensor.matmul(
                                vps,
                                lhsT=xts[dc][kb // 4][
                                    :, 128 * (kb % 4) : 128 * (kb % 4) + 128
                                ],
                                rhs=w_g[4 + vp][:, dc, :],
                                start=(dc == 0), stop=(dc == NDC - 1),
                            )
                        if VT2_SINGLE_COPY:
                            nc.vector.tensor_copy(
                                out=vt2[vp][:, kb, :, 1:65],
                                in_=vps.rearrange("p (two f) -> p two f", two=2),
                            )
                        else:
                            nc.vector.tensor_copy(
                                out=vt2[vp][:, kb, 0, 1:65], in_=vps[:, 0:64]
                            )
                            nc.vector.tensor_copy(
                                out=vt2[vp][:, kb, 1, 1:65], in_=vps[:, 64:128]
                            )
                    out.append(tv)
            return out

        # ---------- attention ----------
        # blocks[(slot, qb)] -> list of (e_tile, col_off, kb)
        blocks = {}

        def qk_thunks(sg, slot_order=None):
            """Pack this s-group's score units into PSUM score tiles.

            Units: ('w', slot, kb) 512-wide prefix matmuls spanning the 4
            q-blocks; ('n', slot, qb, kb) 128-wide band blocks.  Units are
            slot-major (matching projection-copy completion order) with
            each slot's diagonal band blocks packed last and adjacent so
            one affine_select masks a whole run.
            """
            units = []
            for slot in slot_order or range(HPC):
                P, A = slot_pats[slot]
                for kb in range(min(P, 4 * sg + 4, NKB)):
                    units.append(("w", slot, kb))
                if A < 0:
                    continue
                # band blocks merged by k-block: one matmul covers the
                # contiguous q-blocks [q0, q1] attending kb (diag at col 0
                # when kb >= 4sg)
                for kb in range(max(P, 4 * sg - A), 4 * sg + 4):
                    q0 = max(kb, 4 * sg)
                    q1 = min(kb + A, 4 * sg + 3)
                    if q1 < q0:
                        continue
                    units.append(("b", slot, kb, q0, q1 - q0 + 1))

            # greedy fill of QK_TILE-wide tiles; matmuls must not cross a
            # PSUM bank boundary, so a unit that would cross closes the tile
            tiles, cur, cols = [], [], 0
            for u in units:
                if u[0] == "w":
                    w = 512 - 128 * max(0, u[2] - 4 * sg)
                else:
                    w = 128 * u[4]
                if cols + w > QK_TILE:
                    tiles.append(cur)
                    cur, cols = [], 0
                cur.append((u, cols, w))
                cols += w
            if cur:
                tiles.append(cur)

            thunks = []
            for ti, tu in enumerate(tiles):
                holder = {}
                for (u, c0, w) in tu:
                    def t_mm(u=u, c0=c0, w=w, sg=sg, holder=holder, ti=ti):
                        if "qk" not in holder:
                            holder["qk"] = qk_pool.tile(
                                [128, QK_TILE], F32, tag="qk",
                                name=f"qk_{sg}_{ti}",
                            )
                        qk_t = holder["qk"]
                        if u[0] == "w":
                            _, slot, kb = u
                            off = 128 * max(0, kb - 4 * sg)
                            nc.tensor.matmul(
                                qk_t[:, c0 : c0 + w],
                                lhsT=kt[slot][:, 128 * kb : 128 * kb + 128],
                                rhs=qt[slot][:, 512 * sg + off : 512 * sg + 512],
                                start=True, stop=True,
                            )
                        else:
                            _, slot, kb, q0, nq = u
                            nc.tensor.matmul(
                                qk_t[:, c0 : c0 + w],
                                lhsT=kt[slot][:, 128 * kb : 128 * kb + 128],
                                rhs=qt[slot][:, 128 * q0 : 128 * (q0 + nq)],
                                start=True, stop=True,
                            )
                    thunks.append(t_mm)

                def t_exp(tu=tu, sg=sg, holder=holder, ti=ti):
                    qk_t = holder["qk"]
                    ncols = tu[-1][1] + tu[-1][2]
                    e_t = e_pool.tile(
                        [128, QK_TILE], BF16, tag="e", name=f"e_{sg}_{ti}"
                    )
                    nc.scalar.activation(
                        out=e_t[:, 0:ncols], in_=qk_t[:, 0:ncols],
                        func=AF.Exp, scale=1.0 / SCALE32,
                    )
                    # causal masks: one affine_select per run of
                    # same-stride diagonal blocks (zero-stride outer dim)
                    def mask_diag(dcol, stride, nrun):
                        if nrun == 1:
                            pat, ap = [[1, 128]], e_t[:, dcol : dcol + 128]
                        else:
                            pat = [[0, nrun], [1, 128]]
                            ap = e_t[:, dcol : dcol + stride * nrun].rearrange(
                                "p (n c) -> p n c", n=nrun
                            )[:, :, 0:128]
                        nc.gpsimd.affine_select(
                            out=ap, in_=ap,
                            compare_op=mybir.AluOpType.is_ge,
                            fill=0.0, base=0,
                            pattern=pat, channel_multiplier=-1,
                        )

                    run0, rstride, nrun = None, 0, 0
                    for (u, c0, w) in tu:
                        isdiag = (u[0] == "w" and u[2] >= 4 * sg) or (
                            u[0] == "b" and u[3] == u[2]
                        )
                        if not isdiag:
                            continue
                        if (
                            GROUPED_AFFINE
                            and nrun
                            and c0 == run0 + rstride * nrun
                            and u[0] == "b"
                            and w == rstride
                        ):
                            nrun += 1
                            continue
                        if nrun:
                            mask_diag(run0, rstride, nrun)
                        run0, rstride, nrun = c0, w, 1
                    if nrun:
                        mask_diag(run0, rstride, nrun)
                    # register blocks for PV
                    for (u, c0, w) in tu:
                        if u[0] == "w":
                            _, slot, kb = u
                            off = max(0, kb - 4 * sg)
                            for qb in range(4 * sg + off, 4 * sg + 4):
                                blocks.setdefault((slot, qb), []).append(
                                    (e_t, c0 + 128 * (qb - 4 * sg - off), kb)
                                )
                        else:
                            _, slot, kb, q0, nq = u
                            for qi in range(nq):
                                blocks.setdefault((slot, q0 + qi), []).append(
                                    (e_t, c0 + 128 * qi, kb)
                                )
                thunks.append(t_exp)
            return thunks

        def pv_thunks(slot, sg):
            vp, par = slot // 2, slot % 2
            holder = {}
            thunks = []
            for qi in range(4):
                def t(slot=slot, sg=sg, qi=qi):
                    if "pvq" not in holder:
                        holder["pvq"] = pv_pool.tile(
                            [128, 4, 65], F32, tag="pv", name=f"pv_{slot}_{sg}"
                        )
                    pvq = holder["pvq"]
                    qb = 4 * sg + qi
                    blist = blocks.pop((slot, qb))
                    n = len(blist)
                    for bi, (e_t, c0, kb) in enumerate(blist):
                        nc.tensor.matmul(
                            pvq[:, qi, :],
                            lhsT=e_t[:, c0 : c0 + 128],
                            rhs=vt2[vp][:, kb, par, :],
                            start=(bi == 0), stop=(bi == n - 1),
                        )
                thunks.append(t)

            def t_norm(slot=slot, sg=sg):
                pvq = holder["pvq"]
                recip = n_pool.tile(
                    [128, 4], F32, tag="recip", name=f"recip_{slot}_{sg}"
                )
                nc.vector.reciprocal(recip, pvq[:, :, 0])
                r0 = (4 * sg) % 8
                nc.vector.tensor_tensor(
                    out=out_stage[:, r0 : r0 + 4, 64 * slot : 64 * slot + 64],
                    in0=pvq[:, :, 1:65],
                    in1=recip[:, :, None].broadcast_to([128, 4, 64]),
                    op=OP.mult,
                )
            thunks.append(t_norm)
            return thunks

        def interleave(stream, fill):
            nf, nq = len(fill), len(stream)
            fi = 0
            for qi_, tq in enumerate(stream):
                tq()
                want = (qi_ + 1) * nf // max(nq, 1)
                while fi < want:
                    fill[fi]()
                    fi += 1
            while fi < nf:
                fill[fi]()
                fi += 1

        # ---------- prologue ----------
        if DUMMY_EXP:
            # exp table prefetch: tiny activation at t=0 so the ~2.7us
            # ACT_TABLE_LOAD overlaps the input DMAs.
            nc.gpsimd.memset(dummy, 0.0)
            nc.scalar.activation(out=dummy, in_=dummy, func=AF.Exp, scale=1.0)

        # DMA queue order mirrors consumption order (queues are FIFO and a
        # late weight group cascades through the PSUM pool FIFO): w0 first,
        # then interleave the remaining w groups among the sg0 x^T chunks,
        # extras (needed only when QK starts) last.
        if not PAD_EXTRAS:
            # zero the unused contraction rows once (partition base must be
            # 32-aligned, so clear 64:128 first and let the extras DMA then
            # overwrite rows 64:70).  Split across Vector/GpSimd so the
            # prologue memsets don't serialize on one engine.
            ms_engines = [nc.vector, nc.gpsimd]
            for s in range(HPC):
                ms_engines[s % 2].memset(qt[s][64:128, :], 0.0)
                kc = kt[s].shape[-1]
                ms_engines[(s + 1) % 2].memset(kt[s][64:128, 0:kc], 0.0)
        emit_w_dma(0)
        for dc in range(4):
            nc.sync.dma_start(
                out=xts[dc][0],
                in_=xt_ext[128 * dc : 128 * dc + 128, 0:512],
            )
        emit_w_dma(1)
        for dc in range(4, NDC):
            nc.sync.dma_start(
                out=xts[dc][0],
                in_=xt_ext[128 * dc : 128 * dc + 128, 0:512],
            )
        emit_w_dma(2)
        emit_w_dma(3)
        for s in range(HPC):
            emit_extras(s)
        emit_w_dma(4)
        emit_w_dma(5)
        emit_xt_dma(1)
        for t in proj_thunks(0, cp=copy_mix):
            t()
        emit_kprefix()
        emit_xt_dma(2)
        emit_xt_dma(3)

        # ---------- main loop ----------
        prev_pv = []
        for sg in range(4):
            fill = list(prev_pv) + v_thunks(sg)
            if sg < 3:
                fill += proj_thunks(sg + 1)
            so = [2, 3, 1, 0] if sg == 3 else None
            interleave(qk_thunks(sg, slot_order=so), fill)
            if sg > 0:
                for qi in range(4):
                    qb = 4 * (sg - 1) + qi
                    nc.sync.dma_start(
                        out=out_ext[128 * qb : 128 * qb + 128, :],
                        in_=out_stage[:, qb % 8, :],
                    )
            prev_pv = []
            if sg < 3:
                for slot in range(HPC):
                    prev_pv.extend(pv_thunks(slot, sg))
        # tail: PV of the final sg in stream order (each slot's exps are
        # complete by the time its PV chains issue)
        for slot in [2, 3, 1, 0]:
            for t in pv_thunks(slot, 3):
                t()
        for qi in range(4):
            qb = 12 + qi
            nc.sync.dma_start(
                out=out_ext[128 * qb : 128 * qb + 128, :],
                in_=out_stage[:, qb % 8, :],
            )

    persist.release()


def _plan(m_all):
    return _group_heads(np.asarray(m_all, dtype=np.float32).reshape(H))


def _split3(v):
    """Host bf16 hi/lo/lo2 split of an fp32 vector, matching on-chip RNE."""
    bf = ml_dtypes.bfloat16
    hi = v.astype(bf)
    r1 = v - hi.astype(np.float32)
    lo = r1.astype(bf)
    lo2 = (r1 - lo.astype(np.float32)).astype(bf)
    return hi, lo, lo2


def _extras(mv, heads):
    """eq/ek [HPC, nex, S] bf16 ALiBi extras rows (zero-padded if PAD_EXTRAS)."""
    bf = ml_dtypes.bfloat16
    nex = 64 if PAD_EXTRAS else 6
    i = np.arange(S, dtype=np.float32)
    eq = np.zeros((HPC, nex, S), dtype=bf)
    ek = np.zeros((HPC, nex, S), dtype=bf)
    for p, hh in enumerate(heads):
        mh = float(mv[hh])
        vq = (-max(mh, 0.0) * i - C_STAB) * SCALE32
        vk = (mh * i) * SCALE32
        eq[p, 0:3] = np.stack(_split3(vq))
        eq[p, 3:6] = np.ones((3, S), dtype=bf)
        ek[p, 0:3] = np.ones((3, S), dtype=bf)
        ek[p, 3:6] = np.stack(_split3(vk))
    return eq, ek


def _shard_inputs(x, W_kqv, m, cols):
    """Per-core input maps. Core c: batch c//4; slot p runs head cols[p][c%4]."""
    bf = ml_dtypes.bfloat16
    x = np.asarray(x, dtype=np.float32).astype(bf)
    xt = [np.ascontiguousarray(x[b].T) for b in range(B)]
    W = np.asarray(W_kqv, dtype=np.float32).astype(bf)
    mv = np.asarray(m, dtype=np.float32).reshape(H)
    in_maps = []
    ex_cache = {}
    for c in range(N_CORES):
        b, g = c // 4, c % 4
        heads = [cols[p][g] for p in range(HPC)]
        # column groups: [Q0 K0 | Q1 K1 | Q2 Q3 | K2 K3 | V0 V1 | V2 V3]
        def qcol(h):
            return W[:, 1024 + h * 64 : 1024 + h * 64 + 64]
        def kcol(h):
            return W[:, 0 + h * 64 : 0 + h * 64 + 64]
        def vcol(h):
            return W[:, 2048 + h * 64 : 2048 + h * 64 + 64]
        grps = [
            np.concatenate([qcol(heads[0]), kcol(heads[0])], axis=1),
            np.concatenate([qcol(heads[1]), kcol(heads[1])], axis=1),
            np.concatenate([qcol(heads[2]), qcol(heads[3])], axis=1),
            np.concatenate([kcol(heads[2]), kcol(heads[3])], axis=1),
            np.concatenate([vcol(heads[0]), vcol(heads[1])], axis=1),
            np.concatenate([vcol(heads[2]), vcol(heads[3])], axis=1),
        ]
        # [D=1024, 6, 128] -> [128(p), 6, NDC, 128]: p = d % 128, dc = d // 128
        w_all = np.stack(grps, axis=1)  # [1024, 6, 128]
        w_local = np.ascontiguousarray(
            w_all.reshape(NDC, 128, 6, 128).transpose(1, 2, 0, 3)
        )
        if tuple(heads) not in ex_cache:
            ex_cache[tuple(heads)] = _extras(mv, heads)
        eq, ek = ex_cache[tuple(heads)]
        in_maps.append({"xt": xt[b], "w": w_local, "eq": eq, "ek": ek})
    return in_maps


def _run(inputs, trace=False):
    cols, slot_pats = _plan(inputs["m"])
    if slot_pats not in _NC_CACHE:
        _NC_CACHE[slot_pats] = _build_nc(slot_pats)
    nc = _NC_CACHE[slot_pats]
    in_maps = _shard_inputs(inputs["x"], inputs["W_kqv"], inputs["m"], cols)
    res = run_bass_kernel_spmd(
        nc, in_maps, core_ids=list(range(N_CORES)), trace=trace
    )
    out = np.zeros((B, S, D), dtype=np.float32)
    for c in range(N_CORES):
        b, g = c // 4, c % 4
        core_out = np.asarray(res.results[c]["out"], dtype=np.float32)
        for p in range(HPC):
            hh = cols[p][g]
            out[b, :, 64 * hh : 64 * hh + 64] = core_out[:, 64 * p : 64 * p + 64]
    return out, res


def kernel(**inputs) -> np.ndarray:
    out, _ = _run(inputs, trace=False)
    return out


# revision 39
# speedup vs baseline: 1.0286x; 1.0286x over previous
"""ALiBi multi-head causal attention on 8 TRN2 NeuronCores.

Sharding: core c handles batch b = c // 4 and a group of 4 heads chosen by
the host. Fully data/head-parallel: no collectives; host scatters inputs and
concatenates per-core outputs.

ALiBi block sparsity: for slope m, softmax weights vanish once
|m * (j - i)| exceeds MARGIN. Each head only needs a prefix of key blocks
(m < 0: j < P*128) and/or a diagonal band (m > 0: i - 128*A <= j <= i).
The host groups the 16 heads into 4 NEFF slot positions (union pattern per
slot, one head per slot on each core) and compiles a block-sparse schedule
(cached per pattern).

Host prep: x is pre-transposed per batch (xT [D, S]) so the kernel never
transposes on-chip; W is packed per-core into 128-col groups
[Q0K0 | Q1K1 | Q2Q3 | K2K3 | V0V1 | V2V3] so K of prefix-only slots (2,3)
is projected only for the k-blocks those slots attend (kb < P).

Per-core device algorithm (bf16 operands, fp32 PSUM accumulation):
  - Projection transposed per group: pps = (W-group)^T @ x^T, rows 0:64 and
    64:128 copied (Vector/GpSimd) into qt/kt feature tiles.
  - Scores in transposed layout sT[j, i]: prefix k-blocks as 512-wide
    matmuls spanning 4 q-blocks, band blocks as 128-wide units; units are
    packed into 2-bank PSUM tiles so a single ScalarE exp covers <=1024
    cols. The softmax exponent q.k/32 + m*(j-i) - C is produced by the QK
    matmul via six extra contraction rows (bf16 hi/lo/lo2 triples):
       qt rows 64-66 = split(-relu(m)*i - C), rows 67-69 = 1
       kt rows 64-66 = 1,                     rows 67-69 = split(m*j)
    so exp() needs no max pass.
  - Causal mask: diagonal units are packed adjacently so one gpsimd
    affine_select covers a run of them (post-exp, fill 0).
  - PV in natural layout: out[q, hd] accumulated in PSUM with e-blocks
    stationary; V pair-tiles [128, kb, 2, 65] carry a ones column at
    feature 0 so PV col 0 is the softmax denominator; batch-reciprocal +
    per-block scale into a bf16 staging tile, DMA out (host upcasts).
"""

import ml_dtypes
import numpy as np

import concourse.bass as bass
import concourse.mybir as mybir
import concourse.tile as tile
from concourse import bacc
from concourse.bass_utils import run_bass_kernel_spmd

F32 = mybir.dt.float32
BF16 = mybir.dt.bfloat16

B, S, D, H, HD = 2, 2048, 1024, 16, 64
HPC = 4  # heads per core
N_CORES = 8
C_STAB = 8.0
SCALE32 = 32.0
NDC = D // 128  # 8 contraction chunks
NKB = S // 128  # 16 k-blocks
NQB = S // 128  # 16 q-blocks
MARGIN = 12.0  # exponent cutoff for block skipping (weights < e^-12 dropped)
QK_TILE = 512  # score cols per PSUM score tile / per exp

# NOTE: score PSUM tiles must be single-bank (QK_TILE=512): 2-bank tiles
# with multiple start=True matmuls per bank corrupt values on hardware
# (sim-clean, HW-NaN under profiling timing).
# NOTE: tile dependency tracking is whole-tile granular, so x^T and W live
# in per-chunk tiles (a single big tile makes the first matmul wait for
# every DMA issued before it).
PAD_EXTRAS = False  # host-pad eq/ek to 64 rows (no on-chip memset of qt/kt)
DUMMY_EXP = True  # prefetch exp table via dummy activation at t=0
VT2_SINGLE_COPY = True  # single strided copy into V pair tile
GROUPED_AFFINE = True  # one affine_select per run of adjacent diag blocks

_NC_CACHE = {}


def _head_pattern(mh):
    """(P, A): prefix blocks (m<0) and band-back blocks (m>0); A=-1 none."""
    am = abs(float(mh))
    if am < MARGIN / S:
        return (NKB, -1) if mh < 0 else (0, NKB)
    nb = int(np.ceil(MARGIN / am / 128.0))
    if mh < 0:
        return (min(NKB, nb), -1)
    return (0, min(NKB, nb))


def _kb_set(pat, qb):
    P, A = pat
    s = set()
    if P > 0:
        s |= set(range(0, min(qb, P - 1) + 1))
    if A >= 0:
        s |= set(range(max(0, qb - A), qb + 1))
    return sorted(s)


def _pat_cost(pat):
    return sum(len(_kb_set(pat, qb)) for qb in range(NQB))


def _union(pats):
    P = max(p for p, _ in pats) if pats else 0
    A = max(a for _, a in pats) if pats else -1
    return (P, A)


def _kv_blocks(pat):
    need = set()
    for qb in range(NQB):
        need |= set(_kb_set(pat, qb))
    return need


def _grouping_score(cols, pats):
    ups = [_union([pats[x] for x in c]) for c in cols]
    atten = sum(_pat_cost(u) for u in ups)
    order = sorted(range(4), key=lambda ci: -_pat_cost(ups[ci]))
    ups_o = [ups[i] for i in order]
    vp = len(_kv_blocks(ups_o[0]) | _kv_blocks(ups_o[1])) + len(
        _kv_blocks(ups_o[2]) | _kv_blocks(ups_o[3])
    )
    return atten + 2 * vp


def _group_heads(m16):
    """Group 16 heads into 4 slot columns of 4, minimizing union cost.
    Returns (columns, slot_pats) with columns sorted heavy-first."""
    pats = [_head_pattern(m16[h]) for h in range(H)]

    cands = []
    # (i) greedy by attention delta
    order = sorted(range(H), key=lambda h: -_pat_cost(pats[h]))
    cols = [[] for _ in range(4)]
    for h in order:
        best, bestd = None, None
        for ci in range(4):
            if len(cols[ci]) >= 4:
                continue
            cur = _pat_cost(_union([pats[x] for x in cols[ci]])) if cols[ci] else 0
            new = _pat_cost(_union([pats[x] for x in cols[ci]] + [pats[h]]))
            d = new - cur
            if bestd is None or d < bestd or (
                d == bestd and len(cols[ci]) < len(cols[best])
            ):
                best, bestd = ci, d
        cols[best].append(h)
    cands.append([list(c) for c in cols])
    # (ii) structured by sign
    negs = sorted([h for h in range(H) if pats[h][1] < 0], key=lambda h: pats[h][0])
    poss = sorted([h for h in range(H) if pats[h][1] >= 0], key=lambda h: -pats[h][1])
    seq = negs[: (len(negs) // 4) * 4]
    cols2 = [seq[i * 4 : (i + 1) * 4] for i in range(len(seq) // 4)]
    rest = negs[(len(negs) // 4) * 4 :] + poss
    for i in range(0, len(rest), 4):
        cols2.append(rest[i : i + 4])
    if len(cols2) == 4 and all(len(c) == 4 for c in cols2):
        cands.append(cols2)
    best_cols = min(cands, key=lambda c: _grouping_score(c, pats))
    improved = True
    while improved:
        improved = False
        sc0 = _grouping_score(best_cols, pats)
        for a in range(4):
            for b in range(a + 1, 4):
                for i in range(4):
                    for j in range(4):
                        best_cols[a][i], best_cols[b][j] = (
                            best_cols[b][j], best_cols[a][i],
                        )
                        sc = _grouping_score(best_cols, pats)
                        if sc < sc0:
                            sc0 = sc
                            improved = True
                        else:
                            best_cols[a][i], best_cols[b][j] = (
                                best_cols[b][j], best_cols[a][i],
                            )
    cols = best_cols
    slot_pats = [_union([pats[x] for x in c]) for c in cols]
    idx = sorted(range(4), key=lambda ci: -_pat_cost(slot_pats[ci]))
    cols = [cols[i] for i in idx]
    slot_pats = [slot_pats[i] for i in idx]
    return cols, tuple(slot_pats)


def _narrow_blocks(slot_pats, slot, sg):
    """Band blocks of an s-group not covered by the wide prefix units."""
    P, _ = slot_pats[slot]
    out = []
    for qb in range(4 * sg, 4 * sg + 4):
        for kb in _kb_set(slot_pats[slot], qb):
            if kb < P:
                continue  # covered by wide prefix unit
            out.append((qb, kb))
    return out


def _build_nc(slot_pats):
    nc = bacc.Bacc(None, target_bir_lowering=False, debug=False)
    nex = 64 if PAD_EXTRAS else 6
    xt_ext = nc.declare_dram_parameter("xt", [D, S], BF16, isOutput=False)
    w_ext = nc.declare_dram_parameter("w", [128, 6, NDC, 128], BF16, isOutput=False)
    eq_ext = nc.declare_dram_parameter("eq", [HPC, nex, S], BF16, isOutput=False)
    ek_ext = nc.declare_dram_parameter("ek", [HPC, nex, S], BF16, isOutput=False)
    out_ext = nc.declare_dram_parameter("out", [S, HPC * HD], BF16, isOutput=True)

    with tile.TileContext(nc) as tc:
        _emit(nc, tc, xt_ext, w_ext, eq_ext, ek_ext, out_ext, slot_pats)
    nc.finalize()
    return nc


def _emit(nc, tc, xt_ext, w_ext, eq_ext, ek_ext, out_ext, slot_pats):
    AF = mybir.ActivationFunctionType
    OP = mybir.AluOpType

    persist = tc.alloc_tile_pool(name="persist", bufs=1, space="SBUF")

    # --- persistent SBUF tiles ---
    # per-group W and per-(dc, sg) x^T tiles: fine-grained DMA deps
    w_g = [
        persist.tile([128, NDC, 128], BF16, tag=f"w{g}", name=f"w{g}")
        for g in range(6)
    ]
    xts = [
        [
            persist.tile([128, 512], BF16, tag=f"xt{dc}_{sg}", name=f"xt{dc}_{sg}")
            for sg in range(4)
        ]
        for dc in range(NDC)
    ]
    out_stage = persist.tile([128, 8, HPC * HD], BF16, tag="out_stage")

    # wide-prefix slots (P>=1) and per-slot K extent
    P_of = [slot_pats[s][0] for s in range(HPC)]
    full_k = [s for s in range(HPC) if slot_pats[s][1] >= 0 or P_of[s] >= NKB]
    # K tiles: full-band slots need all S cols; prefix-only slots need P*128
    qt, kt = [], []
    for s in range(HPC):
        qt_s = persist.tile([128, S], BF16, tag=f"qt{s}", name=f"qt{s}")
        kcols = S if s in full_k else max(1, P_of[s]) * 128
        kt_s = persist.tile([128, kcols], BF16, tag=f"kt{s}", name=f"kt{s}")
        qt.append(qt_s)
        kt.append(kt_s)

    # V pair tiles: [128, kb, parity, 65]; feature col 0 is the ones column
    # (softmax denominator accumulator), cols 1:65 the 64 head features.
    slot_kv = [_kv_blocks(slot_pats[s]) for s in range(HPC)]
    pair_kv = [sorted(slot_kv[0] | slot_kv[1]), sorted(slot_kv[2] | slot_kv[3])]
    vt2 = []
    for vp in range(2):
        nkb = max(pair_kv[vp]) + 1
        v_t = persist.tile([128, nkb, 2, 65], BF16, tag=f"vt{vp}", name=f"vt{vp}")
        nc.gpsimd.memset(v_t[:, :, :, 0:1], 1.0)
        vt2.append(v_t)

    dummy = persist.tile([128, 8], F32, tag="dummy")

    # PSUM -> SBUF copies: GpSimd cannot access PSUM.  Steady-state copies
    # ride VectorE; the sg0 prologue copies ride the then-idle ScalarE.
    def copy_any(out, in_):
        nc.vector.tensor_copy(out=out, in_=in_)

    def copy_scalar(out, in_):
        nc.scalar.copy(out=out, in_=in_)

    mix_state = [0]

    def copy_mix(out, in_):
        # prologue copies alternate Scalar/Vector so each PSUM tile is
        # released after ~one copy latency instead of two in FIFO
        if mix_state[0] % 2 == 0:
            nc.scalar.copy(out=out, in_=in_)
        else:
            nc.vector.tensor_copy(out=out, in_=in_)
        mix_state[0] += 1

    with (
        tc.tile_pool(name="pp_ps", bufs=3, space="PSUM") as pp_pool,
        tc.tile_pool(name="qk_ps", bufs=3, space="PSUM") as qk_pool,
        tc.tile_pool(name="pv_ps", bufs=2, space="PSUM") as pv_pool,
        tc.tile_pool(
            name="e_pool", bufs=10 * 1024 // QK_TILE, space="SBUF"
        ) as e_pool,
        tc.tile_pool(name="n_pool", bufs=4, space="SBUF") as n_pool,
    ):
        # ---------- DMA emitters ----------
        def emit_w_dma(g):
            nc.sync.dma_start(out=w_g[g], in_=w_ext[:, g])

        def emit_xt_dma(sg):
            for dc in range(NDC):
                nc.sync.dma_start(
                    out=xts[dc][sg],
                    in_=xt_ext[128 * dc : 128 * dc + 128, 512 * sg : 512 * sg + 512],
                )

        nex = 64 if PAD_EXTRAS else 6

        def emit_extras(s):
            nc.sync.dma_start(out=qt[s][64 : 64 + nex, :], in_=eq_ext[s])
            kc = kt[s].shape[-1]
            nc.sync.dma_start(
                out=kt[s][64 : 64 + nex, 0:kc], in_=ek_ext[s, :, 0:kc]
            )

        # ---------- projection ----------
        # full-sequence projection groups: Q always; K-halves of groups 0,1
        # only when that slot attends everywhere; group 3 (K2 K3) only when
        # one of slots 2,3 does.
        proj_groups = [0, 1, 2] + (
            [3] if (2 in full_k or 3 in full_k) else []
        )

        def proj_thunks(sg, cp=None):
            """Per s-group projection of the full-sequence column groups."""
            cp = cp or copy_any
            thunks = []
            for g in proj_groups:
                holder = {}
                for dc in range(NDC):
                    def t_mm(g=g, sg=sg, holder=holder, dc=dc):
                        if "pps" not in holder:
                            holder["pps"] = pp_pool.tile(
                                [128, 512], F32, tag="pp", name=f"pps_{g}_{sg}"
                            )
                        nc.tensor.matmul(
                            holder["pps"],
                            lhsT=w_g[g][:, dc, :],
                            rhs=xts[dc][sg],
                            start=(dc == 0), stop=(dc == NDC - 1),
                        )
                    thunks.append(t_mm)

                def t_cp(g=g, sg=sg, holder=holder, cp=cp):
                    pps = holder["pps"]
                    c0 = 512 * sg
                    if g < 2:
                        cp(qt[g][0:64, c0 : c0 + 512], pps[0:64, :])
                        if g in full_k:
                            cp(kt[g][0:64, c0 : c0 + 512], pps[64:128, :])
                    elif g == 2:
                        cp(qt[2][0:64, c0 : c0 + 512], pps[0:64, :])
                        cp(qt[3][0:64, c0 : c0 + 512], pps[64:128, :])
                    else:
                        for s in (2, 3):
                            if s in full_k:
                                cp(
                                    kt[s][0:64, c0 : c0 + 512],
                                    pps[64 * (s - 2) : 64 * (s - 2) + 64, :],
                                )
                thunks.append(t_cp)
            return thunks

        def emit_kprefix():
            """K for prefix-only slots: only their first P k-blocks."""
            # group 3 carries K2|K3; groups 0,1 carry their own K-half
            jobs = {}  # (grp, kb) -> [(slot, rows0)]
            for s in range(HPC):
                if s in full_k:
                    continue
                g = s if s < 2 else 3
                r0 = 64 if s < 2 else 64 * (s - 2)
                for kb in range(P_of[s]):
                    jobs.setdefault((g, kb), []).append((s, r0))
            for (g, kb), dests in sorted(jobs.items()):
                pps = pp_pool.tile([128, 128], F32, tag="pp", name=f"kp_{g}_{kb}")
                for dc in range(NDC):
                    nc.tensor.matmul(
                        pps,
                        lhsT=w_g[g][:, dc, :],
                        rhs=xts[dc][kb // 4][:, 128 * (kb % 4) : 128 * (kb % 4) + 128],
                        start=(dc == 0), stop=(dc == NDC - 1),
                    )
                for (s, r0) in dests:
                    copy_scalar(
                        kt[s][0:64, 128 * kb : 128 * kb + 128],
                        pps[r0 : r0 + 64, :],
                    )

        # ---------- V projection ----------
        def v_thunks(sg):
            out = []
            for vp in range(2):
                for kb in range(4 * sg, 4 * sg + 4):
                    if kb not in pair_kv[vp]:
                        continue

                    def tv(vp=vp, kb=kb):
                        vps = pp_pool.tile(
                            [128, 128], F32, tag="pp", name=f"vps_{vp}_{kb}"
                        )
                        for dc in range(NDC):
                            nc.tensor.matmul(
                                vps,
                                lhsT=xts[dc][kb // 4][
                                    :, 128 * (kb % 4) : 128 * (kb % 4) + 128
                                ],
                                rhs=w_g[4 + vp][:, dc, :],
                                start=(dc == 0), stop=(dc == NDC - 1),
                            )
                        if VT2_SINGLE_COPY:
                            nc.vector.tensor_copy(
                                out=vt2[vp][:, kb, :, 1:65],
                                in_=vps.rearrange("p (two f) -> p two f", two=2),
                            )
                        else:
                            nc.vector.tensor_copy(
                                out=vt2[vp][:, kb, 0, 1:65], in_=vps[:, 0:64]
                            )
                            nc.vector.tensor_copy(
                                out=vt2[vp][:, kb, 1, 1:65], in_=vps[:, 64:128]
                            )
                    out.append(tv)
            return out

        # ---------- attention ----------
        # blocks[(slot, qb)] -> list of (e_tile, col_off, kb)
        blocks = {}

        def qk_thunks(sg, slot_order=None):
            """Pack this s-group's score units into PSUM score tiles.

            Units: ('w', slot, kb) 512-wide prefix matmuls spanning the 4
            q-blocks; ('n', slot, qb, kb) 128-wide band blocks.  Units are
            slot-major (matching projection-copy completion order) with
            each slot's diagonal band blocks packed last and adjacent so
            one affine_select masks a whole run.
            """
            units = []
            for slot in slot_order or range(HPC):
                P, A = slot_pats[slot]
                for kb in range(min(P, 4 * sg + 4, NKB)):
                    units.append(("w", slot, kb))
                if A < 0:
                    continue
                # band blocks merged by k-block: one matmul covers the
                # contiguous q-blocks [q0, q1] attending kb (diag at col 0
                # when kb >= 4sg)
                for kb in range(max(P, 4 * sg - A), 4 * sg + 4):
                    q0 = max(kb, 4 * sg)
                    q1 = min(kb + A, 4 * sg + 3)
                    if q1 < q0:
                        continue
                    units.append(("b", slot, kb, q0, q1 - q0 + 1))

            # greedy fill of QK_TILE-wide tiles; matmuls must not cross a
            # PSUM bank boundary, so a unit that would cross closes the tile
            tiles, cur, cols = [], [], 0
            for u in units:
                if u[0] == "w":
                    w = 512 - 128 * max(0, u[2] - 4 * sg)
                else:
                    w = 128 * u[4]
                if cols + w > QK_TILE:
                    tiles.append(cur)
                    cur, cols = [], 0
                cur.append((u, cols, w))
                cols += w
            if cur:
                tiles.append(cur)

            thunks = []
            for ti, tu in enumerate(tiles):
                holder = {}
                for (u, c0, w) in tu:
                    def t_mm(u=u, c0=c0, w=w, sg=sg, holder=holder, ti=ti):
                        if "qk" not in holder:
                            holder["qk"] = qk_pool.tile(
                                [128, QK_TILE], F32, tag="qk",
                                name=f"qk_{sg}_{ti}",
                            )
                        qk_t = holder["qk"]
                        if u[0] == "w":
                            _, slot, kb = u
                            off = 128 * max(0, kb - 4 * sg)
                            nc.tensor.matmul(
                                qk_t[:, c0 : c0 + w],
                                lhsT=kt[slot][:, 128 * kb : 128 * kb + 128],
                                rhs=qt[slot][:, 512 * sg + off : 512 * sg + 512],
                                start=True, stop=True,
                            )
                        else:
                            _, slot, kb, q0, nq = u
                            nc.tensor.matmul(
                                qk_t[:, c0 : c0 + w],
                                lhsT=kt[slot][:, 128 * kb : 128 * kb + 128],
                                rhs=qt[slot][:, 128 * q0 : 128 * (q0 + nq)],
                                start=True, stop=True,
                            )
                    thunks.append(t_mm)

                def t_exp(tu=tu, sg=sg, holder=holder, ti=ti):
                    qk_t = holder["qk"]
                    ncols = tu[-1][1] + tu[-1][2]
                    e_t = e_pool.tile(
                        [128, QK_TILE], BF16, tag="e", name=f"e_{sg}_{ti}"
                    )
                    nc.scalar.activation(
                        out=e_t[:, 0:ncols], in_=qk_t[:, 0:ncols],
                        func=AF.Exp, scale=1.0 / SCALE32,
                    )
                    # causal masks: one affine_select per run of
                    # same-stride diagonal blocks (zero-stride outer dim)
                    def mask_diag(dcol, stride, nrun):
                        if nrun == 1:
                            pat, ap = [[1, 128]], e_t[:, dcol : dcol + 128]
                        else:
                            pat = [[0, nrun], [1, 128]]
                            ap = e_t[:, dcol : dcol + stride * nrun].rearrange(
                                "p (n c) -> p n c", n=nrun
                            )[:, :, 0:128]
                        nc.gpsimd.affine_select(
                            out=ap, in_=ap,
                            compare_op=mybir.AluOpType.is_ge,
                            fill=0.0, base=0,
                            pattern=pat, channel_multiplier=-1,
                        )

                    run0, rstride, nrun = None, 0, 0
                    for (u, c0, w) in tu:
                        isdiag = (u[0] == "w" and u[2] >= 4 * sg) or (
                            u[0] == "b" and u[3] == u[2]
                        )
                        if not isdiag:
                            continue
                        if (
                            GROUPED_AFFINE
                            and nrun
                            and c0 == run0 + rstride * nrun
                            and u[0] == "b"
                            and w == rstride
                        ):
                            nrun += 1
                            continue
                        if nrun:
                            mask_diag(run0, rstride, nrun)
                        run0, rstride, nrun = c0, w, 1
                    if nrun:
                        mask_diag(run0, rstride, nrun)
                    # register blocks for PV
                    for (u, c0, w) in tu:
                        if u[0] == "w":
                            _, slot, kb = u
                            off = max(0, kb - 4 * sg)
                            for qb in range(4 * sg + off, 4 * sg + 4):
                                blocks.setdefault((slot, qb), []).append(
                                    (e_t, c0 + 128 * (qb - 4 * sg - off), kb)
                                )
                        else:
                            _, slot, kb, q0, nq = u
                            for qi in range(nq):
                                blocks.setdefault((slot, q0 + qi), []).append(
                                    (e_t, c0 + 128 * qi, kb)
                                )
                thunks.append(t_exp)
            return thunks

        def pv_thunks(slot, sg):
            vp, par = slot // 2, slot % 2
            holder = {}
            thunks = []
            for qi in range(4):
                def t(slot=slot, sg=sg, qi=qi):
                    if "pvq" not in holder:
                        holder["pvq"] = pv_pool.tile(
                            [128, 4, 65], F32, tag="pv", name=f"pv_{slot}_{sg}"
                        )
                    pvq = holder["pvq"]
                    qb = 4 * sg + qi
                    blist = blocks.pop((slot, qb))
                    n = len(blist)
                    for bi, (e_t, c0, kb) in enumerate(blist):
                        nc.tensor.matmul(
                            pvq[:, qi, :],
                            lhsT=e_t[:, c0 : c0 + 128],
                            rhs=vt2[vp][:, kb, par, :],
                            start=(bi == 0), stop=(bi == n - 1),
                        )
                thunks.append(t)

            def t_norm(slot=slot, sg=sg):
                pvq = holder["pvq"]
                recip = n_pool.tile(
                    [128, 4], F32, tag="recip", name=f"recip_{slot}_{sg}"
                )
                nc.vector.reciprocal(recip, pvq[:, :, 0])
                r0 = (4 * sg) % 8
                nc.vector.tensor_tensor(
                    out=out_stage[:, r0 : r0 + 4, 64 * slot : 64 * slot + 64],
                    in0=pvq[:, :, 1:65],
                    in1=recip[:, :, None].broadcast_to([128, 4, 64]),
                    op=OP.mult,
                )
            thunks.append(t_norm)
            return thunks

        def interleave(stream, fill):
            nf, nq = len(fill), len(stream)
            fi = 0
            for qi_, tq in enumerate(stream):
                tq()
                want = (qi_ + 1) * nf // max(nq, 1)
                while fi < want:
                    fill[fi]()
                    fi += 1
            while fi < nf:
                fill[fi]()
                fi += 1

        # ---------- prologue ----------
        if DUMMY_EXP:
            # exp table prefetch: tiny activation at t=0 so the ~2.7us
            # ACT_TABLE_LOAD overlaps the input DMAs.
            nc.gpsimd.memset(dummy, 0.0)
            nc.scalar.activation(out=dummy, in_=dummy, func=AF.Exp, scale=1.0)

        # DMA queue order mirrors consumption order (queues are FIFO and a
        # late weight group cascades through the PSUM pool FIFO): w0 first,
        # then interleave the remaining w groups among the sg0 x^T chunks,
        # extras (needed only when QK starts) last.
        if not PAD_EXTRAS:
            # zero the unused contraction rows once (partition base must be
            # 32-aligned, so clear 64:128 first and let the extras DMA then
            # overwrite rows 64:70).  Split across Vector/GpSimd so the
            # prologue memsets don't serialize on one engine.
            ms_engines = [nc.vector, nc.gpsimd]
            for s in range(HPC):
                ms_engines[s % 2].memset(qt[s][64:128, :], 0.0)
                kc = kt[s].shape[-1]
                ms_engines[(s + 1) % 2].memset(kt[s][64:128, 0:kc], 0.0)
        emit_w_dma(0)
        for dc in range(4):
            nc.sync.dma_start(
                out=xts[dc][0],
                in_=xt_ext[128 * dc : 128 * dc + 128, 0:512],
            )
        emit_w_dma(1)
        for dc in range(4, NDC):
            nc.sync.dma_start(
                out=xts[dc][0],
                in_=xt_ext[128 * dc : 128 * dc + 128, 0:512],
            )
        emit_w_dma(2)
        emit_w_dma(3)
        for s in range(HPC):
            emit_extras(s)
        emit_w_dma(4)
        emit_w_dma(5)
        emit_xt_dma(1)
        for t in proj_thunks(0, cp=copy_mix):
            t()
        emit_kprefix()
        emit_xt_dma(2)
        emit_xt_dma(3)

        # ---------- main loop ----------
        prev_pv = []
        for sg in range(4):
            fill = list(prev_pv) + v_thunks(sg)
            if sg < 3:
                fill += proj_thunks(sg + 1)
            so = [2, 3, 1, 0] if sg == 3 else None
            interleave(qk_thunks(sg, slot_order=so), fill)
            if sg > 0:
                for qi in range(4):
                    qb = 4 * (sg - 1) + qi
                    nc.sync.dma_start(
                        out=out_ext[128 * qb : 128 * qb + 128, :],
                        in_=out_stage[:, qb % 8, :],
                    )
            prev_pv = []
            if sg < 3:
                for slot in range(HPC):
                    prev_pv.extend(pv_thunks(slot, sg))
        # tail: PV of the final sg in stream order (each slot's exps are
        # complete by the time its PV chains issue)
        for slot in [2, 3, 1, 0]:
            for t in pv_thunks(slot, 3):
                t()
        for qi in range(4):
            qb = 12 + qi
            nc.sync.dma_start(
                out=out_ext[128 * qb : 128 * qb + 128, :],
                in_=out_stage[:, qb % 8, :],
            )

    persist.release()


def _plan(m_all):
    return _group_heads(np.asarray(m_all, dtype=np.float32).reshape(H))


def _split3(v):
    """Host bf16 hi/lo/lo2 split of an fp32 vector, matching on-chip RNE."""
    bf = ml_dtypes.bfloat16
    hi = v.astype(bf)
    r1 = v - hi.astype(np.float32)
    lo = r1.astype(bf)
    lo2 = (r1 - lo.astype(np.float32)).astype(bf)
    return hi, lo, lo2


def _extras(mv, heads):
    """eq/ek [HPC, nex, S] bf16 ALiBi extras rows (zero-padded if PAD_EXTRAS)."""
    bf = ml_dtypes.bfloat16
    nex = 64 if PAD_EXTRAS else 6
    i = np.arange(S, dtype=np.float32)
    eq = np.zeros((HPC, nex, S), dtype=bf)
    ek = np.zeros((HPC, nex, S), dtype=bf)
    for p, hh in enumerate(heads):
        mh = float(mv[hh])
        vq = (-max(mh, 0.0) * i - C_STAB) * SCALE32
        vk = (mh * i) * SCALE32
        eq[p, 0:3] = np.stack(_split3(vq))
        eq[p, 3:6] = np.ones((3, S), dtype=bf)
        ek[p, 0:3] = np.ones((3, S), dtype=bf)
        ek[p, 3:6] = np.stack(_split3(vk))
    return eq, ek


def _shard_inputs(x, W_kqv, m, cols):
    """Per-core input maps. Core c: batch c//4; slot p runs head cols[p][c%4]."""
    bf = ml_dtypes.bfloat16
    x = np.asarray(x, dtype=np.float32).astype(bf)
    xt = [np.ascontiguousarray(x[b].T) for b in range(B)]
    W = np.asarray(W_kqv, dtype=np.float32).astype(bf)
    mv = np.asarray(m, dtype=np.float32).reshape(H)
    in_maps = []
    ex_cache = {}
    for c in range(N_CORES):
        b, g = c // 4, c % 4
        heads = [cols[p][g] for p in range(HPC)]
        # column groups: [Q0 K0 | Q1 K1 | Q2 Q3 | K2 K3 | V0 V1 | V2 V3]
        def qcol(h):
            return W[:, 1024 + h * 64 : 1024 + h * 64 + 64]
        def kcol(h):
            return W[:, 0 + h * 64 : 0 + h * 64 + 64]
        def vcol(h):
            return W[:, 2048 + h * 64 : 2048 + h * 64 + 64]
        grps = [
            np.concatenate([qcol(heads[0]), kcol(heads[0])], axis=1),
            np.concatenate([qcol(heads[1]), kcol(heads[1])], axis=1),
            np.concatenate([qcol(heads[2]), qcol(heads[3])], axis=1),
            np.concatenate([kcol(heads[2]), kcol(heads[3])], axis=1),
            np.concatenate([vcol(heads[0]), vcol(heads[1])], axis=1),
            np.concatenate([vcol(heads[2]), vcol(heads[3])], axis=1),
        ]
        # [D=1024, 6, 128] -> [128(p), 6, NDC, 128]: p = d % 128, dc = d // 128
        w_all = np.stack(grps, axis=1)  # [1024, 6, 128]
        w_local = np.ascontiguousarray(
            w_all.reshape(NDC, 128, 6, 128).transpose(1, 2, 0, 3)
        )
        if tuple(heads) not in ex_cache:
            ex_cache[tuple(heads)] = _extras(mv, heads)
        eq, ek = ex_cache[tuple(heads)]
        in_maps.append({"xt": xt[b], "w": w_local, "eq": eq, "ek": ek})
    return in_maps


def _run(inputs, trace=False):
    cols, slot_pats = _plan(inputs["m"])
    if slot_pats not in _NC_CACHE:
        _NC_CACHE[slot_pats] = _build_nc(slot_pats)
    nc = _NC_CACHE[slot_pats]
    in_maps = _shard_inputs(inputs["x"], inputs["W_kqv"], inputs["m"], cols)
    res = run_bass_kernel_spmd(
        nc, in_maps, core_ids=list(range(N_CORES)), trace=trace
    )
    out = np.zeros((B, S, D), dtype=np.float32)
    for c in range(N_CORES):
        b, g = c // 4, c % 4
        core_out = np.asarray(res.results[c]["out"], dtype=np.float32)
        for p in range(HPC):
            hh = cols[p][g]
            out[b, :, 64 * hh : 64 * hh + 64] = core_out[:, 64 * p : 64 * p + 64]
    return out, res


def kernel(**inputs) -> np.ndarray:
    out, _ = _run(inputs, trace=False)
    return out
